# revision 22
# baseline (speedup 1.0000x reference)
"""OHEM-balanced BCE loss (nn_BCELoss_75411035783735) on 8 Trainium2 cores.

reference semantics:
    positive = (gt*mask) > 0 ; negative = ((1-gt)*mask) > 0
    negative_count = min(negative.sum(), floor(positive.sum()*3))
    loss = bce_with_logits(pred_logits, gt)
    out = (sum(loss*positive) + sum(top_k(loss*negative, negative_count)))
          / (positive_count + negative_count + 1e-6)

gt/mask are iid 0/1 here, so negative.sum() <= 3*positive.sum() (checked at
runtime from the B/C partials below): the top-k selects *all* negatives (every
negative BCE term is strictly positive), and the loss collapses to masked
streaming reductions. Using softplus(-x) = softplus(x) - x:
    bce(x, g) = softplus((1-2g)*x) = softplus(x) - x*g          (exact)
so with A1 = sum(softplus(x)*m), A2 = sum(x*g*m), B = sum(g*m), C = sum(m):
    out = (A1 - A2) / (C + 1e-6)

Per core (1/8 of the batch = 1.64M elements = ~19.7MB of HBM reads; the kernel
is DMA-bound, ~310GB/s/core practical):
  Sync:    ONE interleaved DMA per tile (x|g|m packed on the host) -- big
           transfers amortize the ~2us per-DMA completion latency. The tile
           schedule is uneven: a small first tile so compute starts early,
           small last tile so the post-last-byte compute tail is short.
  Vector:  w = g*m via scalar_tensor_tensor whose accum_out gives B for free;
           A2 = sum(x*w); A1 = sum(softplus*m)   (3 passes)
  Scalar:  softplus(x) = Ln(1 + Exp(x)) (2 passes; no Softplus act table in
           this neuronxcc) + C = sum(m) via Identity accum_out (1 pass)
Per-tile partials go straight out via the result DMA (no on-device fold: an
STT accum_out written by instruction N is not readable by instruction N+1 on
the same engine -- observed accumulator write-back race). Host sums 8x128x4K
partials in f64; a host fallback computes exact reference semantics if the
top-k ever failed to degenerate (C-B > floor(3B)).
"""

from contextlib import ExitStack

import numpy as np

import concourse.bass as bass
import concourse.mybir as mybir
from concourse.bass_utils import run_bass_kernel_spmd

N_CORES = 8
P = 128
SHAPE = (32, 640, 640)
TOTAL = SHAPE[0] * SHAPE[1] * SHAPE[2]
PER_CORE = TOTAL // N_CORES  # 1,638,400
FREE = PER_CORE // P  # 12,800 elements per partition per core

# Uneven tile schedule (sums to FREE): small head tile -> compute starts after
# ~1.6MB instead of ~3.9MB; small tail tile -> short serial epilogue.
TILES = [1024, 2560, 2560, 2560, 2560, 1536]
assert sum(TILES) == FREE
K_TILES = len(TILES)
F_MAX = max(TILES)
NBUF = 3  # input-stream buffers (xgmt); w/expo/sp stay double-buffered
CBUF = 2

_BUILT = None  # cached Bass module across calls


def _build_nc():
    f32 = mybir.dt.float32
    AF = mybir.ActivationFunctionType
    ALU = mybir.AluOpType

    nc = bass.Bass(
        "TRN2",
        debug=False,
        enable_asserts=False,
        target_bir_lowering=False,
        num_devices=N_CORES,
    )
    xgm_d = nc.dram_tensor("xgm", [3 * PER_CORE], f32, kind="ExternalInput").ap()
    o_d = nc.dram_tensor(
        "partials", [P, 4 * K_TILES], f32, kind="ExternalOutput"
    ).ap()

    K = K_TILES
    F3 = 3 * F_MAX
    # DRAM offset of each packed tile (3*P*F elements per tile)
    offs = np.cumsum([0] + [3 * P * f for f in TILES]).tolist()

    with (
        nc.sbuf_tensor([P, NBUF * F3], f32) as xgmt,
        nc.sbuf_tensor([P, CBUF * F_MAX], f32) as wt,
        nc.sbuf_tensor([P, CBUF * F_MAX], f32) as expo,
        nc.sbuf_tensor([P, CBUF * F_MAX], f32) as sp,
        # one [P, 4K] block of per-tile partials: A1 | A2 | B | C columns
        nc.sbuf_tensor([P, 4 * K_TILES], f32) as accs,
        nc.sbuf_tensor([P, 1], f32) as dum_v,
        nc.sbuf_tensor([P, 1], f32) as dum_s,
        ExitStack() as _sem_stack,
        nc.semaphore() as v_sem,
        nc.semaphore() as s_sem,
        nc.Block(no_gpsimd_drain=True) as block,
    ):
        # One dedicated semaphore per input tile: a shared counter is NOT a
        # completion indicator -- the +16 arrives as per-SDMA-engine incs of 1
        # (16 slots/load), so sem >= 16*(i+1) can be met while a lagging slot
        # of load i is still in flight (observed: partition-group-aligned
        # stale reads under profiling). sem_i >= 16 is unambiguous.
        dma_ld = [
            _sem_stack.enter_context(nc.semaphore(name=f"dma_ld{i}"))
            for i in range(K_TILES)
        ]
        acc1 = accs[:, 0 * K : 1 * K]
        acc2 = accs[:, 1 * K : 2 * K]
        accb = accs[:, 2 * K : 3 * K]
        accc = accs[:, 3 * K : 4 * K]

        # x/g/m slices of the packed tile in buffer b for tile i
        def xs(b, i):
            return xgmt[:, b * F3 + 0 * TILES[i] : b * F3 + 1 * TILES[i]]

        def gs(b, i):
            return xgmt[:, b * F3 + 1 * TILES[i] : b * F3 + 2 * TILES[i]]

        def ms(b, i):
            return xgmt[:, b * F3 + 2 * TILES[i] : b * F3 + 3 * TILES[i]]

        # per-iteration increments: dma +16, v +3 (w/B, A2, A1), s +2 (ln, C)

        @block.sync
        def _(sync):
            for i in range(K):
                b = i % NBUF
                f = TILES[i]
                if i >= NBUF:
                    sync.wait_ge(v_sem, 3 * (i - NBUF) + 3)  # V.A1_{i-NBUF} done
                    sync.wait_ge(s_sem, 2 * (i - NBUF) + 2)  # S.C_{i-NBUF} done
                src = xgm_d[offs[i] : offs[i + 1]].rearrange(
                    "(t p f) -> p t f", t=3, p=P
                )
                dst = xgmt[:, b * F3 : b * F3 + 3 * f].rearrange(
                    "p (t f) -> p t f", t=3
                )
                sync.dma_start(dst, src).then_inc(dma_ld[i], 16)
            sync.wait_ge(v_sem, 3 * K)      # V.A1_{K-1} accum landed
            sync.wait_ge(s_sem, 2 * K + 1)  # S accum fence retired
            sync.dma_start(o_d[:], accs[:]).then_inc(dma_ld[0], 16)

        @block.scalar
        def _(scalar):
            for i in range(K):
                b = i % NBUF
                b2 = i % CBUF
                f = TILES[i]
                scalar.wait_ge(dma_ld[i], 16)
                if i >= CBUF:
                    # WAR: sp[b2] consumed by V.A1_{i-CBUF}
                    scalar.wait_ge(v_sem, 3 * (i - CBUF) + 3)
                nc.scalar.activation(
                    expo[:, b2 * F_MAX : b2 * F_MAX + f], xs(b, i), AF.Exp
                )
                nc.scalar.activation(
                    sp[:, b2 * F_MAX : b2 * F_MAX + f],
                    expo[:, b2 * F_MAX : b2 * F_MAX + f], AF.Ln, bias=1.0,
                ).then_inc(s_sem, 1)
                # C partial: sum(mask)
                nc.scalar.activation(
                    dum_s.ap().broadcast_to((P, f)), ms(b, i), AF.Identity,
                    accum_out=accc[:, i : i + 1],
                ).then_inc(s_sem, 1)
            # Fence: activation accum_out lowers to ACTIVATE +
            # ACTIVATION_READ_ACCUMULATOR; the sem inc rides the ACTIVATE, so
            # accc[:, K-1] may not be committed when s_sem hits 2K. This
            # in-order no-op retires after the accumulator read; its inc
            # gates the result DMA.
            nc.scalar.copy(dum_s[:], dum_s[:]).then_inc(s_sem, 1)

        @block.vector
        def _(vector):
            for i in range(K):
                b = i % NBUF
                b2 = i % CBUF
                f = TILES[i]
                vector.wait_ge(dma_ld[i], 16)
                # w = g*m, and its accum gives B = sum(g*m) for free
                nc.vector.scalar_tensor_tensor(
                    wt[:, b2 * F_MAX : b2 * F_MAX + f], gs(b, i), 1.0, ms(b, i),
                    op0=ALU.mult, op1=ALU.mult, accum_out=accb[:, i : i + 1],
                ).then_inc(v_sem, 1)
                # A2 partial: sum(x*w) = sum(x*g*m)
                nc.vector.scalar_tensor_tensor(
                    dum_v.ap().broadcast_to((P, f)), xs(b, i), 1.0,
                    wt[:, b2 * F_MAX : b2 * F_MAX + f],
                    op0=ALU.mult, op1=ALU.mult, accum_out=acc2[:, i : i + 1],
                ).then_inc(v_sem, 1)
                # A1 partial: sum(softplus(x)*m)
                vector.wait_ge(s_sem, 2 * i + 1)
                nc.vector.scalar_tensor_tensor(
                    dum_v.ap().broadcast_to((P, f)),
                    sp[:, b2 * F_MAX : b2 * F_MAX + f], 1.0, ms(b, i),
                    op0=ALU.mult, op1=ALU.mult, accum_out=acc1[:, i : i + 1],
                ).then_inc(v_sem, 1)

    return nc


def _pack_inputs(pred_logits, gt, mask):
    """Pack x|g|m per core into the uneven-tile interleaved stream."""
    x = np.ascontiguousarray(pred_logits, dtype=np.float32).reshape(N_CORES, P, FREE)
    g = np.ascontiguousarray(gt, dtype=np.float32).reshape(N_CORES, P, FREE)
    m = np.ascontiguousarray(mask, dtype=np.float32).reshape(N_CORES, P, FREE)
    out = np.empty((N_CORES, 3 * PER_CORE), dtype=np.float32)
    off = 0
    col = 0
    for f in TILES:
        n = P * f
        for t, a in enumerate((x, g, m)):
            out[:, off + t * n : off + (t + 1) * n] = a[
                :, :, col : col + f
            ].reshape(N_CORES, n)
        off += 3 * n
        col += f
    return out


def _reference_fallback(pred_logits, gt, mask):
    # Exact (host) replica of the reference for the non-degenerate top-k case.
    x = pred_logits.astype(np.float64)
    g = gt.astype(np.float64)
    m = mask.astype(np.float64)
    positive = (g * m) > 0
    negative = ((1.0 - g) * m) > 0
    pos_count = int(positive.sum())
    neg_cap = int(np.float32(pos_count) * np.float32(3.0))
    neg_count = min(int(negative.sum()), neg_cap)
    loss = np.maximum(x, 0.0) - x * g + np.log1p(np.exp(-np.abs(x)))
    pos_sum = (loss * positive).sum()
    neg_losses = loss[negative]
    if neg_count < neg_losses.size:
        top = np.partition(neg_losses, neg_losses.size - neg_count)[
            neg_losses.size - neg_count :
        ]
    else:
        top = neg_losses
    denom = pos_count + neg_count + 1e-6
    return np.float32((pos_sum + top.sum()) / denom)


def kernel(pred_logits, gt, mask):
    global _BUILT
    assert pred_logits.shape == SHAPE and gt.shape == SHAPE and mask.shape == SHAPE
    if _BUILT is None:
        _BUILT = _build_nc()
    nc = _BUILT

    xgm = _pack_inputs(pred_logits, gt, mask)
    in_maps = [{"xgm": xgm[c]} for c in range(N_CORES)]
    res = run_bass_kernel_spmd(nc, in_maps, core_ids=list(range(N_CORES)))

    K = K_TILES
    a1 = a2 = b = c = 0.0
    for r in res.results:
        p = r["partials"].astype(np.float64)
        a1 += p[:, 0 * K : 1 * K].sum()
        a2 += p[:, 1 * K : 2 * K].sum()
        b += p[:, 2 * K : 3 * K].sum()
        c += p[:, 3 * K : 4 * K].sum()

    a = a1 - a2
    pos_count = int(round(b))
    total_count = int(round(c))
    neg_count = total_count - pos_count
    neg_cap = int(np.float32(pos_count) * np.float32(3.0))
    if neg_count > neg_cap:
        return _reference_fallback(pred_logits, gt, mask)
    return np.float32(a / (pos_count + neg_count + 1e-6))


# revision 24
# speedup vs baseline: 1.0008x; 1.0008x over previous
"""OHEM-balanced BCE loss (nn_BCELoss_75411035783735) on 8 Trainium2 cores.

reference semantics:
    positive = (gt*mask) > 0 ; negative = ((1-gt)*mask) > 0
    negative_count = min(negative.sum(), floor(positive.sum()*3))
    loss = bce_with_logits(pred_logits, gt)
    out = (sum(loss*positive) + sum(top_k(loss*negative, negative_count)))
          / (positive_count + negative_count + 1e-6)

gt/mask are iid 0/1 here, so negative.sum() <= 3*positive.sum() (checked at
runtime from the B/C partials below): the top-k selects *all* negatives (every
negative BCE term is strictly positive), and the loss collapses to masked
streaming reductions. Using softplus(-x) = softplus(x) - x:
    bce(x, g) = softplus((1-2g)*x) = softplus(x) - x*g          (exact)
so with A1 = sum(softplus(x)*m), A2 = sum(x*g*m), B = sum(g*m), C = sum(m):
    out = (A1 - A2) / (C + 1e-6)

Per core (1/8 of the batch = 1.64M elements = ~19.7MB of HBM reads; the kernel
is DMA-bound, ~310GB/s/core practical):
  Sync:    ONE interleaved DMA per tile (x|g|m packed on the host) -- big
           transfers amortize the ~2us per-DMA completion latency. The tile
           schedule is uneven: a small first tile so compute starts early,
           small last tile so the post-last-byte compute tail is short.
  Vector:  w = g*m via scalar_tensor_tensor whose accum_out gives B for free;
           A2 = sum(x*w); A1 = sum(softplus*m)   (3 passes)
  Scalar:  softplus(x) = Ln(1 + Exp(x)) (2 passes; no Softplus act table in
           this neuronxcc) + C = sum(m) via Identity accum_out (1 pass)
Per-tile partials go straight out via the result DMA (no on-device fold: an
STT accum_out written by instruction N is not readable by instruction N+1 on
the same engine -- observed accumulator write-back race). Host sums 8x128x4K
partials in f64; a host fallback computes exact reference semantics if the
top-k ever failed to degenerate (C-B > floor(3B)).
"""

from contextlib import ExitStack

import numpy as np

import concourse.bass as bass
import concourse.mybir as mybir
from concourse.bass_utils import run_bass_kernel_spmd

N_CORES = 8
P = 128
SHAPE = (32, 640, 640)
TOTAL = SHAPE[0] * SHAPE[1] * SHAPE[2]
PER_CORE = TOTAL // N_CORES  # 1,638,400
FREE = PER_CORE // P  # 12,800 elements per partition per core

# Uneven tile schedule (sums to FREE): small head tile -> compute starts after
# ~1.6MB instead of ~3.9MB; small tail tile -> short serial epilogue.
TILES = [1024, 2560, 2560, 2560, 2560, 1536]
assert sum(TILES) == FREE
K_TILES = len(TILES)
F_MAX = max(TILES)
NBUF = 3  # input-stream buffers (xgmt); w/expo/sp stay double-buffered
CBUF = 2

_BUILT = None  # cached Bass module across calls


def _build_nc():
    f32 = mybir.dt.float32
    AF = mybir.ActivationFunctionType
    ALU = mybir.AluOpType

    nc = bass.Bass(
        "TRN2",
        debug=False,
        enable_asserts=False,
        target_bir_lowering=False,
        num_devices=N_CORES,
    )
    xgm_d = nc.dram_tensor("xgm", [3 * PER_CORE], f32, kind="ExternalInput").ap()
    o_d = nc.dram_tensor(
        "partials", [P, 4 * K_TILES], f32, kind="ExternalOutput"
    ).ap()

    K = K_TILES
    F3 = 3 * F_MAX
    # DRAM offset of each packed tile (3*P*F elements per tile)
    offs = np.cumsum([0] + [3 * P * f for f in TILES]).tolist()

    with (
        nc.sbuf_tensor([P, NBUF * F3], f32) as xgmt,
        nc.sbuf_tensor([P, CBUF * F_MAX], f32) as wt,
        nc.sbuf_tensor([P, CBUF * F_MAX], f32) as expo,
        nc.sbuf_tensor([P, CBUF * F_MAX], f32) as sp,
        # one [P, 4K] block of per-tile partials: A1 | A2 | B | C columns
        nc.sbuf_tensor([P, 4 * K_TILES], f32) as accs,
        nc.sbuf_tensor([P, 1], f32) as dum_v,
        nc.sbuf_tensor([P, 1], f32) as dum_s,
        ExitStack() as _sem_stack,
        nc.semaphore() as v_sem,
        nc.semaphore() as s_sem,
        nc.Block(no_gpsimd_drain=True) as block,
    ):
        # One dedicated semaphore per input tile: a shared counter is NOT a
        # completion indicator -- the +16 arrives as per-SDMA-engine incs of 1
        # (16 slots/load), so sem >= 16*(i+1) can be met while a lagging slot
        # of load i is still in flight (observed: partition-group-aligned
        # stale reads under profiling). sem_i >= 16 is unambiguous.
        dma_ld = [
            _sem_stack.enter_context(nc.semaphore(name=f"dma_ld{i}"))
            for i in range(K_TILES)
        ]
        acc1 = accs[:, 0 * K : 1 * K]
        acc2 = accs[:, 1 * K : 2 * K]
        accb = accs[:, 2 * K : 3 * K]
        accc = accs[:, 3 * K : 4 * K]

        # x/g/m slices of the packed tile in buffer b for tile i
        def xs(b, i):
            return xgmt[:, b * F3 + 0 * TILES[i] : b * F3 + 1 * TILES[i]]

        def gs(b, i):
            return xgmt[:, b * F3 + 1 * TILES[i] : b * F3 + 2 * TILES[i]]

        def ms(b, i):
            return xgmt[:, b * F3 + 2 * TILES[i] : b * F3 + 3 * TILES[i]]

        # per-iteration increments: dma +16, v +3 (w/B, A2, A1), s +2 (ln, C)

        @block.sync
        def _(sync):
            for i in range(K):
                b = i % NBUF
                f = TILES[i]
                if i >= NBUF:
                    sync.wait_ge(v_sem, 3 * (i - NBUF) + 3)  # V.A1_{i-NBUF} done
                    sync.wait_ge(s_sem, 2 * (i - NBUF) + 2)  # S.C_{i-NBUF} done
                src = xgm_d[offs[i] : offs[i + 1]].rearrange(
                    "(t p f) -> p t f", t=3, p=P
                )
                dst = xgmt[:, b * F3 : b * F3 + 3 * f].rearrange(
                    "p (t f) -> p t f", t=3
                )
                sync.dma_start(dst, src).then_inc(dma_ld[i], 16)
            sync.wait_ge(v_sem, 3 * K + 1)  # V accum fence retired
            sync.wait_ge(s_sem, 2 * K + 1)  # S accum fence retired
            sync.dma_start(o_d[:], accs[:]).then_inc(dma_ld[0], 16)

        @block.scalar
        def _(scalar):
            for i in range(K):
                b = i % NBUF
                b2 = i % CBUF
                f = TILES[i]
                scalar.wait_ge(dma_ld[i], 16)
                if i >= CBUF:
                    # WAR: sp[b2] consumed by V.A1_{i-CBUF}
                    scalar.wait_ge(v_sem, 3 * (i - CBUF) + 3)
                nc.scalar.activation(
                    expo[:, b2 * F_MAX : b2 * F_MAX + f], xs(b, i), AF.Exp
                )
                nc.scalar.activation(
                    sp[:, b2 * F_MAX : b2 * F_MAX + f],
                    expo[:, b2 * F_MAX : b2 * F_MAX + f], AF.Ln, bias=1.0,
                ).then_inc(s_sem, 1)
                # C partial: sum(mask)
                nc.scalar.activation(
                    dum_s.ap().broadcast_to((P, f)), ms(b, i), AF.Identity,
                    accum_out=accc[:, i : i + 1],
                ).then_inc(s_sem, 1)
            # Fence: activation accum_out lowers to ACTIVATE +
            # ACTIVATION_READ_ACCUMULATOR; the sem inc rides the ACTIVATE, so
            # accc[:, K-1] may not be committed when s_sem hits 2K. This
            # in-order no-op retires after the accumulator read; its inc
            # gates the result DMA.
            nc.scalar.copy(dum_s[:], dum_s[:]).then_inc(s_sem, 1)

        @block.vector
        def _(vector):
            for i in range(K):
                b = i % NBUF
                b2 = i % CBUF
                f = TILES[i]
                vector.wait_ge(dma_ld[i], 16)
                # w = g*m, and its accum gives B = sum(g*m) for free
                nc.vector.scalar_tensor_tensor(
                    wt[:, b2 * F_MAX : b2 * F_MAX + f], gs(b, i), 1.0, ms(b, i),
                    op0=ALU.mult, op1=ALU.mult, accum_out=accb[:, i : i + 1],
                ).then_inc(v_sem, 1)
                # A2 partial: sum(x*w) = sum(x*g*m)
                nc.vector.scalar_tensor_tensor(
                    dum_v.ap().broadcast_to((P, f)), xs(b, i), 1.0,
                    wt[:, b2 * F_MAX : b2 * F_MAX + f],
                    op0=ALU.mult, op1=ALU.mult, accum_out=acc2[:, i : i + 1],
                ).then_inc(v_sem, 1)
                # A1 partial: sum(softplus(x)*m)
                vector.wait_ge(s_sem, 2 * i + 1)
                nc.vector.scalar_tensor_tensor(
                    dum_v.ap().broadcast_to((P, f)),
                    sp[:, b2 * F_MAX : b2 * F_MAX + f], 1.0, ms(b, i),
                    op0=ALU.mult, op1=ALU.mult, accum_out=acc1[:, i : i + 1],
                ).then_inc(v_sem, 1)
            # Fence (same hazard class as the scalar one): make sure the last
            # STT's accum_out write-back has retired before the result DMA.
            nc.vector.tensor_copy(dum_v[:], dum_v[:]).then_inc(v_sem, 1)

    return nc


def _pack_inputs(pred_logits, gt, mask):
    """Pack x|g|m per core into the uneven-tile interleaved stream."""
    x = np.ascontiguousarray(pred_logits, dtype=np.float32).reshape(N_CORES, P, FREE)
    g = np.ascontiguousarray(gt, dtype=np.float32).reshape(N_CORES, P, FREE)
    m = np.ascontiguousarray(mask, dtype=np.float32).reshape(N_CORES, P, FREE)
    out = np.empty((N_CORES, 3 * PER_CORE), dtype=np.float32)
    off = 0
    col = 0
    for f in TILES:
        n = P * f
        for t, a in enumerate((x, g, m)):
            out[:, off + t * n : off + (t + 1) * n] = a[
                :, :, col : col + f
            ].reshape(N_CORES, n)
        off += 3 * n
        col += f
    return out


def _reference_fallback(pred_logits, gt, mask):
    # Exact (host) replica of the reference for the non-degenerate top-k case.
    x = pred_logits.astype(np.float64)
    g = gt.astype(np.float64)
    m = mask.astype(np.float64)
    positive = (g * m) > 0
    negative = ((1.0 - g) * m) > 0
    pos_count = int(positive.sum())
    neg_cap = int(np.float32(pos_count) * np.float32(3.0))
    neg_count = min(int(negative.sum()), neg_cap)
    loss = np.maximum(x, 0.0) - x * g + np.log1p(np.exp(-np.abs(x)))
    pos_sum = (loss * positive).sum()
    neg_losses = loss[negative]
    if neg_count < neg_losses.size:
        top = np.partition(neg_losses, neg_losses.size - neg_count)[
            neg_losses.size - neg_count :
        ]
    else:
        top = neg_losses
    denom = pos_count + neg_count + 1e-6
    return np.float32((pos_sum + top.sum()) / denom)


def kernel(pred_logits, gt, mask):
    global _BUILT
    assert pred_logits.shape == SHAPE and gt.shape == SHAPE and mask.shape == SHAPE
    if _BUILT is None:
        _BUILT = _build_nc()
    nc = _BUILT

    xgm = _pack_inputs(pred_logits, gt, mask)
    in_maps = [{"xgm": xgm[c]} for c in range(N_CORES)]
    res = run_bass_kernel_spmd(nc, in_maps, core_ids=list(range(N_CORES)))

    K = K_TILES
    a1 = a2 = b = c = 0.0
    for r in res.results:
        p = r["partials"].astype(np.float64)
        a1 += p[:, 0 * K : 1 * K].sum()
        a2 += p[:, 1 * K : 2 * K].sum()
        b += p[:, 2 * K : 3 * K].sum()
        c += p[:, 3 * K : 4 * K].sum()

    a = a1 - a2
    pos_count = int(round(b))
    total_count = int(round(c))
    neg_count = total_count - pos_count
    neg_cap = int(np.float32(pos_count) * np.float32(3.0))
    if neg_count > neg_cap:
        return _reference_fallback(pred_logits, gt, mask)
    return np.float32(a / (pos_count + neg_count + 1e-6))
